# revision 1
# baseline (speedup 1.0000x reference)
"""BitLinear (absmean ternary-quantized linear) on 8 TRN2 NeuronCores.

Strategy (tensor-parallel, column sharding):
  - weight [16384, 4096] sharded along out-features: 2048 rows per core.
  - x [4,2048,4096] -> [8192, 4096] replicated to all cores (bf16, pre-blocked
    host-side into PE-stationary tile layout [mb, p, kt, m] so each m-block is
    one 1 MiB DMA with 8 KiB contiguous per partition).
  - absmean scale is global over W: each core computes a per-partition abs-sum
    of its shard, AllReduce(add) across the 8 cores, then a ones-matmul reduces
    across partitions and broadcasts the global sum to all 128 partitions.
  - quantize: wq = (w > T) - (w < -T) with T = 0.5*scale, equivalent to
    clip(round(w/scale), -1, 1) incl. RNE tie behavior; stored bf16 unscaled,
    the fp32 scale is applied in the ScalarE PSUM->SBUF copy.
  - matmul: out[m, n] = sum_k x[m, k] * wq[n, k] via PE: lhsT = x-tile
    [K=128, M=128] stationary, rhs = wq-tile [K=128, N=512] moving, fp32 PSUM.
    Two passes: nb=0 alone first (so the PE starts as soon as the first 32
    quantized chunks exist), then nb=1..3 per m-block (x loaded twice total).
  - engine/queue split: W chunks split in halves across sync+gpsimd queues,
    x loads on sync, quantize+reductions on vector, collective bounce DMAs on
    scalar, PSUM->SBUF copies (with scale) + out stores on scalar.
  - output [8192, 2048] fp32 per core, host concatenates along out-features.
"""

import os
import sys

import numpy as np

sys.path.insert(0, "/opt/trn_rl_repo")

import ml_dtypes  # noqa: E402

from concourse import bacc, mybir, tile  # noqa: E402
from concourse.bass_utils import run_bass_kernel_spmd  # noqa: E402


def _install_ntff_hook_shim():
    """bass_utils' trace path needs antenv.axon_hooks, which this image's
    antenv lacks. Recreate the boot-time hook (see trn_agent_boot/trn_boot.py
    _ntff_profile_via_ctypes) against the axon PJRT .so so NTFF profiling
    (HW exec_time_ns) works."""
    import contextlib
    import ctypes
    import types

    try:
        from antenv.axon_hooks import get_axon_ntff_profile_hook  # noqa: F401

        return  # real module present
    except ImportError:
        pass

    so_path = "/opt/axon/libaxon_pjrt.so"
    if not os.path.exists(so_path):
        return
    lib = ctypes.CDLL(so_path)
    if not hasattr(lib, "axon_start_nrt_profile"):
        return
    lib.axon_start_nrt_profile.argtypes = [
        ctypes.POINTER(ctypes.c_int64),
        ctypes.c_size_t,
    ]
    lib.axon_start_nrt_profile.restype = ctypes.c_int64
    lib.axon_stop_nrt_profile.argtypes = [ctypes.c_char_p]
    lib.axon_stop_nrt_profile.restype = ctypes.c_int64

    @contextlib.contextmanager
    def _hook(output_dir, device_ids):
        import jax

        jax.devices()
        if device_ids:
            ids = (ctypes.c_int64 * len(device_ids))(*device_ids)
            rc = lib.axon_start_nrt_profile(ids, len(device_ids))
        else:
            rc = lib.axon_start_nrt_profile(None, 0)
        if rc != 0:
            raise RuntimeError(f"axon_start_nrt_profile rc={rc}")
        try:
            yield
        finally:
            n = lib.axon_stop_nrt_profile(str(output_dir).encode())
            if n < 0:
                raise RuntimeError(f"axon_stop_nrt_profile rc={n}")

    mod = types.ModuleType("antenv.axon_hooks")
    _state = {"hook": _hook}
    mod.set_axon_ntff_profile_hook = lambda h: _state.__setitem__("hook", h)
    mod.get_axon_ntff_profile_hook = lambda: _state["hook"]
    sys.modules["antenv.axon_hooks"] = mod


_install_ntff_hook_shim()

N_CORES = 8
B, S, K, NF = 4, 2048, 4096, 16384
M = B * S  # 8192 tokens
NL = NF // N_CORES  # 2048 out-features per core
KT = K // 128  # 32 contraction tiles
MB = M // 128  # 64 token blocks
NB = NL // 512  # 4 out-feature chunks of 512
INV_NELEM = 1.0 / (NF * K)

LAST_EXEC_NS = None
LAST_RESULTS = None

_nc_cache = None


def _build_nc():
    f32 = mybir.dt.float32
    bf16 = mybir.dt.bfloat16

    nc = bacc.Bacc(
        "TRN2", target_bir_lowering=False, debug=False, num_devices=N_CORES
    )
    xs = nc.declare_dram_parameter("xs", [MB, 128, KT, 128], bf16, isOutput=False)
    wt = nc.declare_dram_parameter("wt", [NB, KT, 128, 512], f32, isOutput=False)
    out = nc.declare_dram_parameter("out", [M, NL], f32, isOutput=True)

    add = mybir.AluOpType.add
    mult = mybir.AluOpType.mult
    sub = mybir.AluOpType.subtract
    amax = mybir.AluOpType.max
    amin = mybir.AluOpType.min

    with tile.TileContext(nc) as tc:
        with (
            tc.tile_pool(name="wq_pool", bufs=1) as wq_pool,
            tc.tile_pool(name="wstage", bufs=9) as wstage,
            tc.tile_pool(name="tmp_pool", bufs=3) as tmp_pool,
            tc.tile_pool(name="xstage", bufs=3) as xstage,
            tc.tile_pool(name="ostage", bufs=4) as ostage,
            tc.tile_pool(name="small", bufs=1) as small,
            tc.tile_pool(name="psum", bufs=7, space="PSUM") as psum_pool,
            tc.tile_pool(name="dram", bufs=1, space="DRAM") as dram_pool,
        ):
            # Resident quantized weights, one tile per (nb, kt) chunk.
            wq = {}
            for nb in range(NB):
                for kt in range(KT):
                    wq[(nb, kt)] = wq_pool.tile(
                        [128, 512], bf16, name=f"wq_{nb}_{kt}", tag=f"wq_{nb}_{kt}"
                    )

            def load_w_chunk(wst, nb, kt, thirds):
                # Spread each 256 KiB chunk across DMA queues so several DMA
                # engines work on it concurrently (per-engine BW is the
                # pipeline limiter, not issue rate).
                del thirds
                nc.sync.dma_start(wst[:, 0:256], wt[nb, kt, :, 0:256])
                nc.gpsimd.dma_start(wst[:, 256:512], wt[nb, kt, :, 256:512])

            # ---- Phase A: local abs-sum, AllReduce, global scale ----
            partials = small.tile([128, NB * KT], f32, name="partials")
            for nb in range(NB):
                for kt in range(KT):
                    c = nb * KT + kt
                    wst = wstage.tile([128, 512], f32, name="wst", tag="wst")
                    load_w_chunk(wst, nb, kt, thirds=True)
                    nc.vector.tensor_reduce(
                        partials[:, c : c + 1],
                        wst[:],
                        axis=mybir.AxisListType.X,
                        op=add,
                        apply_absolute_value=True,
                    )
            loc = small.tile([128, 1], f32, name="loc")
            nc.vector.tensor_reduce(
                loc[:], partials[:], axis=mybir.AxisListType.X, op=add
            )
            # Bounce DMAs ride the Scalar queue (idle here); keeping them off
            # the w-load queues avoids the readback stalling behind w-issues.
            cc_in = dram_pool.tile([128, 1], f32, name="cc_in")
            cc_out = dram_pool.tile([128, 1], f32, name="cc_out", addr_space="Shared")
            nc.scalar.dma_start(cc_in[:], loc[:])
            with tc.high_priority():
                nc.gpsimd.collective_compute(
                    "AllReduce",
                    add,
                    replica_groups=[list(range(N_CORES))],
                    ins=[cc_in.opt()],
                    outs=[cc_out.opt()],
                )
            ar_sb = small.tile([128, 1], f32, name="ar_sb")
            nc.scalar.dma_start(ar_sb[:], cc_out[:])

            # Reduce across partitions + broadcast: ones[128,128].T @ ar_sb[128,1]
            ones = small.tile([128, 128], f32, name="ones")
            nc.vector.memset(ones[:], 1.0)
            psum_s = psum_pool.tile([128, 1], f32, name="psum_s", tag="mm")
            nc.tensor.matmul(psum_s[:], ones[:], ar_sb[:], start=True, stop=True)

            scale_sb = small.tile([128, 1], f32, name="scale_sb")
            nc.vector.tensor_scalar(
                out=scale_sb[:], in0=psum_s[:],
                scalar1=INV_NELEM, scalar2=1e-5, op0=mult, op1=amax,
            )
            # Quantization thresholds +-T = +-0.5*scale (exact in fp32).
            thr = small.tile([128, 1], f32, name="thr")
            nc.vector.tensor_scalar(
                out=thr[:], in0=scale_sb[:], scalar1=0.5, scalar2=None, op0=mult
            )
            nthr = small.tile([128, 1], f32, name="nthr")
            nc.vector.tensor_scalar(
                out=nthr[:], in0=scale_sb[:], scalar1=-0.5, scalar2=None, op0=mult
            )

            # ---- Phase B: quantize w -> wq = (w > T) - (w < -T) in {-1,0,1},
            # bf16, unscaled (scale is applied in the fp32 PSUM->SBUF copy).
            # Matches clip(round(w/scale), -1, 1): |w/s| >= 0.5 <=> nonzero,
            # and the 1.5 boundary is irrelevant after the clip.
            for nb in range(NB):
                for kt in range(KT):
                    c = nb * KT + kt
                    wst = wstage.tile([128, 512], f32, name="wst", tag="wst")
                    load_w_chunk(wst, nb, kt, thirds=False)
                    t1 = tmp_pool.tile([128, 512], f32, name="t1", tag="t1")
                    # t1 = (w < -T)
                    nc.vector.tensor_scalar(
                        out=t1[:], in0=wst[:],
                        scalar1=nthr[:], scalar2=None,
                        op0=mybir.AluOpType.is_lt,
                    )
                    # wq = (w > T) - t1
                    nc.vector.scalar_tensor_tensor(
                        out=wq[(nb, kt)][:], in0=wst[:],
                        scalar=thr[:], in1=t1[:],
                        op0=mybir.AluOpType.is_gt, op1=sub,
                    )

            # ---- Phase C: out[mb] = x[mb] @ wq.T ----
            # Pass 1: nb=0 only (starts as soon as the first 32 chunks are
            # quantized, giving the quantizer ~540us of PE runway).
            # Pass 2: nb=1..3 per m-block.
            def do_block(mb, nbs):
                xst = xstage.tile([128, KT, 128], bf16, name="xst", tag="xst")
                nc.sync.dma_start(xst[:, :, :], xs[mb])
                for nb in nbs:
                    psum = psum_pool.tile(
                        [128, 512], f32, name=f"ps_{mb}_{nb}", tag="mm"
                    )
                    for kt in range(KT):
                        nc.tensor.matmul(
                            psum[:],
                            xst[:, kt, :],
                            wq[(nb, kt)][:],
                            start=(kt == 0),
                            stop=(kt == KT - 1),
                        )
                    ost = ostage.tile([128, 512], f32, name="ost", tag="ost")
                    # out = psum * scale (fp32), on ScalarE (has a PSUM port)
                    nc.scalar.activation(
                        ost[:],
                        psum[:],
                        mybir.ActivationFunctionType.Copy,
                        scale=scale_sb[:],
                    )
                    nc.scalar.dma_start(
                        out[mb * 128 : (mb + 1) * 128, nb * 512 : (nb + 1) * 512],
                        ost[:],
                    )

            for mb in range(MB):
                do_block(mb, [0])
            for mb in range(MB):
                do_block(mb, [1, 2, 3])

    nc.compile()
    return nc


def _get_nc():
    global _nc_cache
    if _nc_cache is None:
        _nc_cache = _build_nc()
    return _nc_cache


def kernel(x: np.ndarray, weight: np.ndarray) -> np.ndarray:
    global LAST_EXEC_NS, LAST_RESULTS
    x = np.asarray(x, dtype=np.float32)
    weight = np.asarray(weight, dtype=np.float32)

    nc = _get_nc()

    # x -> stationary tile layout [mb, k(part), kt, m], bf16: per (mb, p) the
    # [kt, m] plane is 8 KiB contiguous, so each m-block loads as one DMA.
    xf = x.reshape(M, K)
    xs = xf.reshape(MB, 128, KT, 128).transpose(0, 3, 2, 1)
    xs = np.ascontiguousarray(xs).astype(ml_dtypes.bfloat16)

    in_maps = []
    for c in range(N_CORES):
        wsh = weight[c * NL : (c + 1) * NL, :]  # [2048, 4096]
        # -> [nb, kt, k(part), n] chunks
        wtc = wsh.T.reshape(KT, 128, NB, 512).transpose(2, 0, 1, 3)
        in_maps.append({"xs": xs, "wt": np.ascontiguousarray(wtc)})

    trace = bool(int(os.environ.get("BASS_KERNEL_TRACE", "0")))
    res = run_bass_kernel_spmd(
        nc, in_maps, core_ids=list(range(N_CORES)), trace=trace
    )
    LAST_EXEC_NS = res.exec_time_ns
    LAST_RESULTS = res

    outs = [np.asarray(res.results[c]["out"]) for c in range(N_CORES)]
    full = np.concatenate(outs, axis=1).reshape(B, S, NF).astype(np.float32)
    return full



# revision 3
# speedup vs baseline: 1.3267x; 1.3267x over previous
"""BitLinear (absmean ternary-quantized linear) on 8 TRN2 NeuronCores.

Strategy (tensor-parallel, column sharding, fp8 DoubleRow matmul):
  - weight [16384, 4096] sharded along out-features: 2048 rows per core.
  - absmean scale is global over W: each core computes a per-partition abs-sum
    of its shard (streamed as bf16 to halve the head-phase DMA; the absmean of
    RNE-rounded bf16 weights matches fp32 to ~1e-6 relative, far below the
    threshold's sensitivity), AllReduce(add) across the 8 cores, then a
    ones-matmul reduces across partitions and broadcasts the global sum.
  - quantize: wq = (w > T) - (w < -T) with T = 0.5*scale from the fp32 W
    (equivalent to clip(round(w/scale), -1, 1)); stored as fp8e4 {-1,0,1}
    (exact), unscaled; the fp32 scale is applied in the PSUM->SBUF copy.
  - matmul: fp8e4 DoubleRow perf mode - each instruction contracts K=256
    (two 128-row planes) at double rate: lhsT = x tile [128, 2, 128]
    stationary, rhs = wq slice [128, 2, 512] moving, fp32 PSUM accumulate.
  - precision: e4m3 x alone gives rel err ~0.024 (> 2e-2 gate). A residual
    pass r8 = e4m3(x - e4m3(x)) over the first half of K cancels half the
    quantization noise power -> rel err ~0.017 at 1.5x matmul cost, still
    well ahead of bf16 (which needs 2x the PE cycles of the fp8 main pass).
  - two passes over m-blocks: nb=0 alone first (starts as soon as the first
    quarter of W is quantized, giving the quantizer a long PE runway), then
    nb=1..3 per m-block (stationary x tile shared by the three nb matmuls).
  - engine/queue split: absum W (bf16) + quant W (fp32) loads round-robin on
    sync+scalar, x/r tiles on sync, collective bounce + AllReduce all on
    gpsimd (keeps the scalar queue unblocked), absum+quant math on vector,
    PSUM->SBUF scaled copies on scalar, out stores on gpsimd.
"""

import os
import sys

import numpy as np

sys.path.insert(0, "/opt/trn_rl_repo")

import ml_dtypes  # noqa: E402

from concourse import bacc, mybir, tile  # noqa: E402
from concourse.bass_utils import run_bass_kernel_spmd  # noqa: E402


def _install_ntff_hook_shim():
    """bass_utils' trace path needs antenv.axon_hooks, which this image's
    antenv lacks. Recreate the boot-time hook against the axon PJRT .so so
    NTFF profiling (HW exec_time_ns) works."""
    import contextlib
    import ctypes
    import types

    try:
        from antenv.axon_hooks import get_axon_ntff_profile_hook  # noqa: F401

        return  # real module present
    except ImportError:
        pass

    so_path = "/opt/axon/libaxon_pjrt.so"
    if not os.path.exists(so_path):
        return
    lib = ctypes.CDLL(so_path)
    if not hasattr(lib, "axon_start_nrt_profile"):
        return
    lib.axon_start_nrt_profile.argtypes = [
        ctypes.POINTER(ctypes.c_int64),
        ctypes.c_size_t,
    ]
    lib.axon_start_nrt_profile.restype = ctypes.c_int64
    lib.axon_stop_nrt_profile.argtypes = [ctypes.c_char_p]
    lib.axon_stop_nrt_profile.restype = ctypes.c_int64

    @contextlib.contextmanager
    def _hook(output_dir, device_ids):
        import jax

        jax.devices()
        if device_ids:
            ids = (ctypes.c_int64 * len(device_ids))(*device_ids)
            rc = lib.axon_start_nrt_profile(ids, len(device_ids))
        else:
            rc = lib.axon_start_nrt_profile(None, 0)
        if rc != 0:
            raise RuntimeError(f"axon_start_nrt_profile rc={rc}")
        try:
            yield
        finally:
            n = lib.axon_stop_nrt_profile(str(output_dir).encode())
            if n < 0:
                raise RuntimeError(f"axon_stop_nrt_profile rc={n}")

    mod = types.ModuleType("antenv.axon_hooks")
    _state = {"hook": _hook}
    mod.set_axon_ntff_profile_hook = lambda h: _state.__setitem__("hook", h)
    mod.get_axon_ntff_profile_hook = lambda: _state["hook"]
    sys.modules["antenv.axon_hooks"] = mod


_install_ntff_hook_shim()

N_CORES = 8
B, S, K, NF = 4, 2048, 4096, 16384
M = B * S  # 8192 tokens
NL = NF // N_CORES  # 2048 out-features per core
KT = K // 128  # 32 contraction subtiles of 128
K2 = KT // 2  # 16 DoubleRow chunks of 256
MB = M // 128  # 64 token blocks
NB = NL // 512  # 4 out-feature chunks of 512
KR = KT // 2  # residual covers first 16 subtiles (k < 2048)
K2R = KR // 2  # 8 residual DoubleRow chunks
AT = 32  # absum stream tiles [128, 2048] bf16
QG = KT // 4  # quant quad-groups per nb
INV_NELEM = 1.0 / (NF * K)

LAST_EXEC_NS = None
LAST_RESULTS = None

_nc_cache = None


def _build_nc():
    f32 = mybir.dt.float32
    bf16 = mybir.dt.bfloat16
    f8 = mybir.dt.float8e4

    nc = bacc.Bacc(
        "TRN2", target_bir_lowering=False, debug=False, num_devices=N_CORES
    )
    xs = nc.declare_dram_parameter("xs", [MB, 128, KT, 128], f8, isOutput=False)
    rs = nc.declare_dram_parameter("rs", [MB, 128, KR, 128], f8, isOutput=False)
    wa = nc.declare_dram_parameter("wa", [AT, 128, 2048], bf16, isOutput=False)
    wt = nc.declare_dram_parameter("wt", [NB, 128, KT, 512], f32, isOutput=False)
    out = nc.declare_dram_parameter("out", [M, NL], f32, isOutput=True)

    add = mybir.AluOpType.add
    mult = mybir.AluOpType.mult
    sub = mybir.AluOpType.subtract
    amax = mybir.AluOpType.max
    dr = mybir.MatmulPerfMode.DoubleRow

    with tile.TileContext(nc) as tc:
        with (
            tc.tile_pool(name="wq_pool", bufs=1) as wq_pool,
            tc.tile_pool(name="astage", bufs=4) as astage,
            tc.tile_pool(name="wstage", bufs=4) as wstage,
            tc.tile_pool(name="tmp_pool", bufs=2) as tmp_pool,
            tc.tile_pool(name="xstage", bufs=3) as xstage,
            tc.tile_pool(name="rstage", bufs=3) as rstage,
            tc.tile_pool(name="ostage", bufs=6) as ostage,
            tc.tile_pool(name="small", bufs=1) as small,
            tc.tile_pool(name="psum", bufs=8, space="PSUM") as psum_pool,
            tc.tile_pool(name="dram", bufs=1, space="DRAM") as dram_pool,
        ):
            # Resident quantized weights: [128(k), kt, n] fp8, sliced
            # [:, 2j:2j+2, nb*512:...] as the DoubleRow moving operand.
            wq = wq_pool.tile([128, KT, NL], f8, name="wq", tag="wq")

            # ---- Phase A: local abs-sum over bf16 W, AllReduce, scale ----
            partials = small.tile([128, AT], f32, name="partials")
            for t in range(AT):
                wab = astage.tile([128, 2048], bf16, name="wab", tag="wab")
                eng = nc.sync if t % 2 == 0 else nc.scalar
                eng.dma_start(wab[:], wa[t])
                nc.vector.tensor_reduce(
                    partials[:, t : t + 1],
                    wab[:],
                    axis=mybir.AxisListType.X,
                    op=add,
                    apply_absolute_value=True,
                )
            loc = small.tile([128, 1], f32, name="loc")
            nc.vector.tensor_reduce(
                loc[:], partials[:], axis=mybir.AxisListType.X, op=add
            )
            # Bounce DMAs + collective all on gpsimd: the in-order queue
            # naturally sequences write -> AllReduce -> readback without
            # blocking the W-streaming queues.
            cc_in = dram_pool.tile([128, 1], f32, name="cc_in")
            cc_out = dram_pool.tile([128, 1], f32, name="cc_out", addr_space="Shared")
            nc.gpsimd.dma_start(cc_in[:], loc[:])
            with tc.high_priority():
                nc.gpsimd.collective_compute(
                    "AllReduce",
                    add,
                    replica_groups=[list(range(N_CORES))],
                    ins=[cc_in.opt()],
                    outs=[cc_out.opt()],
                )
            ar_sb = small.tile([128, 1], f32, name="ar_sb")
            nc.gpsimd.dma_start(ar_sb[:], cc_out[:])

            # Reduce across partitions + broadcast: ones[128,128].T @ ar_sb
            ones = small.tile([128, 128], f32, name="ones")
            nc.vector.memset(ones[:], 1.0)
            psum_s = psum_pool.tile([128, 1], f32, name="psum_s", tag="mm")
            nc.tensor.matmul(psum_s[:], ones[:], ar_sb[:], start=True, stop=True)

            scale_sb = small.tile([128, 1], f32, name="scale_sb")
            nc.vector.tensor_scalar(
                out=scale_sb[:], in0=psum_s[:],
                scalar1=INV_NELEM, scalar2=1e-5, op0=mult, op1=amax,
            )
            thr = small.tile([128, 1], f32, name="thr")
            nc.vector.tensor_scalar(
                out=thr[:], in0=scale_sb[:], scalar1=0.5, scalar2=None, op0=mult
            )
            nthr = small.tile([128, 1], f32, name="nthr")
            nc.vector.tensor_scalar(
                out=nthr[:], in0=scale_sb[:], scalar1=-0.5, scalar2=None, op0=mult
            )

            # ---- Phase B: quantize fp32 W -> wq in {-1,0,1} fp8, nb-major so
            # pass 1 (nb=0) can start after a quarter of W is quantized.
            for nb in range(NB):
                for g in range(QG):
                    wst = wstage.tile([128, 4, 512], f32, name="wst", tag="wst")
                    eng = nc.sync if g % 2 == 0 else nc.scalar
                    eng.dma_start(wst[:], wt[nb, :, 4 * g : 4 * g + 4, :])
                    t1 = tmp_pool.tile([128, 4, 512], f32, name="t1", tag="t1")
                    # t1 = (w < -T)
                    nc.vector.tensor_scalar(
                        out=t1[:], in0=wst[:],
                        scalar1=nthr[:], scalar2=None,
                        op0=mybir.AluOpType.is_lt,
                    )
                    # wq = (w > T) - t1
                    nc.vector.scalar_tensor_tensor(
                        out=wq[:, 4 * g : 4 * g + 4, nb * 512 : (nb + 1) * 512],
                        in0=wst[:],
                        scalar=thr[:], in1=t1[:],
                        op0=mybir.AluOpType.is_gt, op1=sub,
                    )

            # ---- Phase C: out[mb] = x[mb] @ wq.T, fp8 DoubleRow ----
            def do_block(mb, nbs):
                xst = xstage.tile([128, KT, 128], f8, name="xst", tag="xst")
                nc.sync.dma_start(xst[:, :, :], xs[mb])
                rst = rstage.tile([128, KR, 128], f8, name="rst", tag="rst")
                nc.sync.dma_start(rst[:, :, :], rs[mb])
                psums = {
                    nb: psum_pool.tile([128, 512], f32, name=f"ps_{mb}_{nb}", tag="mm")
                    for nb in nbs
                }
                for j in range(K2 + K2R):
                    if j < K2:
                        lhs = xst[:, 2 * j : 2 * j + 2, :]
                        jj = j
                    else:
                        jj = j - K2
                        lhs = rst[:, 2 * jj : 2 * jj + 2, :]
                    for nb in nbs:
                        nc.tensor.matmul(
                            psums[nb][:],
                            lhs,
                            wq[:, 2 * jj : 2 * jj + 2, nb * 512 : (nb + 1) * 512],
                            start=(j == 0),
                            stop=(j == K2 + K2R - 1),
                            perf_mode=dr,
                        )
                for nb in nbs:
                    ost = ostage.tile([128, 512], f32, name="ost", tag="ost")
                    # out = psum * scale (fp32), on ScalarE (has a PSUM port)
                    nc.scalar.activation(
                        ost[:],
                        psums[nb][:],
                        mybir.ActivationFunctionType.Copy,
                        scale=scale_sb[:],
                    )
                    nc.gpsimd.dma_start(
                        out[mb * 128 : (mb + 1) * 128, nb * 512 : (nb + 1) * 512],
                        ost[:],
                    )

            for mb in range(MB):
                do_block(mb, [0])
            for mb in range(MB):
                do_block(mb, [1, 2, 3])

    nc.compile()
    return nc


def _get_nc():
    global _nc_cache
    if _nc_cache is None:
        _nc_cache = _build_nc()
    return _nc_cache


def kernel(x: np.ndarray, weight: np.ndarray) -> np.ndarray:
    global LAST_EXEC_NS, LAST_RESULTS
    x = np.asarray(x, dtype=np.float32)
    weight = np.asarray(weight, dtype=np.float32)

    nc = _get_nc()

    f8 = ml_dtypes.float8_e4m3

    # x -> stationary tile layout [mb, k(part), kt, m]: per (mb, p) the
    # [kt, m] plane is contiguous, so each m-block loads as one DMA.
    xf = x.reshape(M, K)
    x8 = xf.astype(f8)
    xsh = np.ascontiguousarray(
        x8.reshape(MB, 128, KT, 128).transpose(0, 3, 2, 1)
    )
    # residual of the fp8 cast, itself in fp8, for the first half of K
    r = (xf - x8.astype(np.float32))[:, : KR * 128].astype(f8)
    rsh = np.ascontiguousarray(r.reshape(MB, 128, KR, 128).transpose(0, 3, 2, 1))

    wbf = weight.astype(ml_dtypes.bfloat16)

    in_maps = []
    for c in range(N_CORES):
        wsh = weight[c * NL : (c + 1) * NL, :]  # [2048, 4096] fp32
        # quant layout [nb, k(part), kt, n]: per partition the (kt, n) plane
        # is contiguous, so a kt-group loads as one descriptor per partition.
        wtc = np.ascontiguousarray(
            wsh.T.reshape(KT, 128, NB, 512).transpose(2, 1, 0, 3)
        )
        # absum layout: any partitioning works, plain reshape is free-ish
        wac = wbf[c * NL : (c + 1) * NL, :].reshape(AT, 128, 2048)
        in_maps.append({"xs": xsh, "rs": rsh, "wa": wac, "wt": wtc})

    trace = bool(int(os.environ.get("BASS_KERNEL_TRACE", "0")))
    res = run_bass_kernel_spmd(
        nc, in_maps, core_ids=list(range(N_CORES)), trace=trace
    )
    LAST_EXEC_NS = res.exec_time_ns
    LAST_RESULTS = res

    outs = [np.asarray(res.results[c]["out"]) for c in range(N_CORES)]
    full = np.concatenate(outs, axis=1).reshape(B, S, NF).astype(np.float32)
    return full


# revision 6
# speedup vs baseline: 1.3666x; 1.0301x over previous
"""BitLinear (absmean ternary-quantized linear) on 8 TRN2 NeuronCores.

Strategy (tensor-parallel, column sharding, fp8 DoubleRow matmul):
  - weight [16384, 4096] sharded along out-features: 2048 rows per core.
  - absmean scale is global over W: each core computes a per-partition abs-sum
    of its shard (streamed as bf16 to halve the head-phase DMA; the absmean of
    RNE-rounded bf16 weights matches fp32 to ~1e-6 relative, far below the
    threshold's sensitivity), AllReduce(add) across the 8 cores, then a
    ones-matmul reduces across partitions and broadcasts the global sum.
  - quantize: wq = (w > T) - (w < -T) with T = 0.5*scale from the fp32 W
    (equivalent to clip(round(w/scale), -1, 1)); stored as fp8e4 {-1,0,1}
    (exact), unscaled; the fp32 scale is applied in the PSUM->SBUF copy.
  - matmul: fp8e4 DoubleRow perf mode - each instruction contracts K=256
    (two 128-row planes) at double rate: lhsT = x tile [128, 2, 128]
    stationary, rhs = wq slice [128, 2, 512] moving, fp32 PSUM accumulate.
  - precision: e4m3 x alone gives rel err ~0.024 (> 2e-2 gate). A residual
    pass r8 = e4m3(x - e4m3(x)) over the first 7/16 of K cancels that much
    of the quantization noise power -> rel err ~0.0179 at 1.44x matmul cost,
    still well ahead of bf16 (2x the PE work of the fp8 main pass).
  - two passes over m-blocks: nb=0 alone first (starts as soon as the first
    quarter of W is quantized, giving the quantizer a long PE runway), then
    nb=1..3 per m-block (stationary x tile shared by the three nb matmuls).
  - engine/ring split: fp32 W stream on the scalar ring from T=0; bf16
    absum W split sync/gpsimd rings, abs-sums split DVE (tensor_reduce) /
    Activation (Abs+accum); x/r tiles on sync (prefetched ahead of pass 1);
    collective bounce + AllReduce on gpsimd; PSUM->SBUF scaled copies on
    scalar; out stores on gpsimd.
"""

import os
import sys

import numpy as np

sys.path.insert(0, "/opt/trn_rl_repo")

import ml_dtypes  # noqa: E402

from concourse import bacc, mybir, tile  # noqa: E402
from concourse.bass_utils import run_bass_kernel_spmd  # noqa: E402


def _install_ntff_hook_shim():
    """bass_utils' trace path needs antenv.axon_hooks, which this image's
    antenv lacks. Recreate the boot-time hook against the axon PJRT .so so
    NTFF profiling (HW exec_time_ns) works."""
    import contextlib
    import ctypes
    import types

    try:
        from antenv.axon_hooks import get_axon_ntff_profile_hook  # noqa: F401

        return  # real module present
    except ImportError:
        pass

    so_path = "/opt/axon/libaxon_pjrt.so"
    if not os.path.exists(so_path):
        return
    lib = ctypes.CDLL(so_path)
    if not hasattr(lib, "axon_start_nrt_profile"):
        return
    lib.axon_start_nrt_profile.argtypes = [
        ctypes.POINTER(ctypes.c_int64),
        ctypes.c_size_t,
    ]
    lib.axon_start_nrt_profile.restype = ctypes.c_int64
    lib.axon_stop_nrt_profile.argtypes = [ctypes.c_char_p]
    lib.axon_stop_nrt_profile.restype = ctypes.c_int64

    @contextlib.contextmanager
    def _hook(output_dir, device_ids):
        import jax

        jax.devices()
        if device_ids:
            ids = (ctypes.c_int64 * len(device_ids))(*device_ids)
            rc = lib.axon_start_nrt_profile(ids, len(device_ids))
        else:
            rc = lib.axon_start_nrt_profile(None, 0)
        if rc != 0:
            raise RuntimeError(f"axon_start_nrt_profile rc={rc}")
        try:
            yield
        finally:
            n = lib.axon_stop_nrt_profile(str(output_dir).encode())
            if n < 0:
                raise RuntimeError(f"axon_stop_nrt_profile rc={n}")

    mod = types.ModuleType("antenv.axon_hooks")
    _state = {"hook": _hook}
    mod.set_axon_ntff_profile_hook = lambda h: _state.__setitem__("hook", h)
    mod.get_axon_ntff_profile_hook = lambda: _state["hook"]
    sys.modules["antenv.axon_hooks"] = mod


_install_ntff_hook_shim()

N_CORES = 8
B, S, K, NF = 4, 2048, 4096, 16384
M = B * S  # 8192 tokens
NL = NF // N_CORES  # 2048 out-features per core
KT = K // 128  # 32 contraction subtiles of 128
K2 = KT // 2  # 16 DoubleRow chunks of 256
MB = M // 128  # 64 token blocks
NB = NL // 512  # 4 out-feature chunks of 512
K2R = 7  # residual DoubleRow chunks (first 7/16 of K; rel err ~0.0179)
KR = 2 * K2R  # residual subtiles (k < 1792)
AT = 32  # absum stream tiles [128, 2048] bf16
QG = KT // 4  # quant quad-groups per nb (8)
NQ = NB * QG  # total quant quad tiles (32)
INV_NELEM = 1.0 / (NF * K)

LAST_EXEC_NS = None
LAST_RESULTS = None

_nc_cache = None


def _build_nc():
    f32 = mybir.dt.float32
    bf16 = mybir.dt.bfloat16
    f8 = mybir.dt.float8e4

    nc = bacc.Bacc(
        "TRN2", target_bir_lowering=False, debug=False, num_devices=N_CORES
    )
    xs = nc.declare_dram_parameter("xs", [MB, 128, KT, 128], f8, isOutput=False)
    rs = nc.declare_dram_parameter("rs", [MB, 128, KR, 128], f8, isOutput=False)
    wa = nc.declare_dram_parameter("wa", [AT, 128, 2048], bf16, isOutput=False)
    wt = nc.declare_dram_parameter("wt", [NB, 128, KT, 512], f32, isOutput=False)
    out = nc.declare_dram_parameter("out", [M, NL], f32, isOutput=True)

    add = mybir.AluOpType.add
    mult = mybir.AluOpType.mult
    sub = mybir.AluOpType.subtract
    amax = mybir.AluOpType.max
    dr = mybir.MatmulPerfMode.DoubleRow

    with tile.TileContext(nc) as tc:
        with (
            tc.tile_pool(name="wq_pool", bufs=1) as wq_pool,
            tc.tile_pool(name="astage", bufs=6) as astage,
            tc.tile_pool(name="wstage", bufs=6) as wstage,
            tc.tile_pool(name="tmp_pool", bufs=2) as tmp_pool,
            tc.tile_pool(name="xstage", bufs=4) as xstage,
            tc.tile_pool(name="rstage", bufs=4) as rstage,
            tc.tile_pool(name="ostage", bufs=6) as ostage,
            tc.tile_pool(name="small", bufs=1) as small,
            tc.tile_pool(name="psum", bufs=8, space="PSUM") as psum_pool,
            tc.tile_pool(name="dram", bufs=1, space="DRAM") as dram_pool,
        ):
            # Resident quantized weights: [128(k), kt, n] fp8, sliced
            # [:, 2j:2j+2, nb*512:...] as the DoubleRow moving operand.
            wq = wq_pool.tile([128, KT, NL], f8, name="wq", tag="wq")

            # ---- Phase A absum tiles + phase B quant-source prefetch.
            # Software-pipelined: DMA lookahead stays below the pool depth so
            # ring-buffer reuse always lands behind an already-emitted reader.
            ALOOK = 4
            atiles = {}

            def a_issue(t):
                wab = astage.tile([128, 2048], bf16, name="wab", tag="wab")
                eng = nc.sync if t % 2 == 0 else nc.gpsimd
                eng.dma_start(wab[:], wa[t])
                atiles[t] = wab

            trash = small.tile([128, 2048], bf16, name="trash")
            partials = small.tile([128, AT], f32, name="partials")

            def a_consume(t):
                wab = atiles.pop(t)
                if t % 2 == 0:
                    nc.vector.tensor_reduce(
                        partials[:, t : t + 1],
                        wab[:],
                        axis=mybir.AxisListType.X,
                        op=add,
                        apply_absolute_value=True,
                    )
                else:
                    nc.scalar.activation(
                        trash[:],
                        wab[:],
                        mybir.ActivationFunctionType.Abs,
                        accum_out=partials[:, t : t + 1],
                    )

            # First chunk of the fp32 W stream goes out on the scalar ring
            # immediately (it is not needed until the quantizer starts).
            WLOOK = 4
            wtiles = {}

            def w_issue(i):
                nb, g = divmod(i, QG)
                wst = wstage.tile([128, 4, 512], f32, name="wst", tag="wst")
                nc.scalar.dma_start(wst[:], wt[nb, :, 4 * g : 4 * g + 4, :])
                wtiles[i] = wst

            for i in range(WLOOK):
                w_issue(i)

            for t in range(ALOOK):
                a_issue(t)
            for t in range(AT):
                if t + ALOOK < AT:
                    a_issue(t + ALOOK)
                a_consume(t)

            loc = small.tile([128, 1], f32, name="loc")
            nc.vector.tensor_reduce(
                loc[:], partials[:], axis=mybir.AxisListType.X, op=add
            )
            # Bounce DMAs + collective all on gpsimd: the in-order queue
            # naturally sequences write -> AllReduce -> readback without
            # blocking the W-streaming queues.
            cc_in = dram_pool.tile([128, 1], f32, name="cc_in")
            cc_out = dram_pool.tile([128, 1], f32, name="cc_out", addr_space="Shared")
            nc.gpsimd.dma_start(cc_in[:], loc[:])
            with tc.high_priority():
                nc.gpsimd.collective_compute(
                    "AllReduce",
                    add,
                    replica_groups=[list(range(N_CORES))],
                    ins=[cc_in.opt()],
                    outs=[cc_out.opt()],
                )
            ar_sb = small.tile([128, 1], f32, name="ar_sb")
            nc.gpsimd.dma_start(ar_sb[:], cc_out[:])

            # Reduce across partitions + broadcast: ones[128,128].T @ ar_sb
            ones = small.tile([128, 128], f32, name="ones")
            nc.vector.memset(ones[:], 1.0)
            psum_s = psum_pool.tile([128, 1], f32, name="psum_s", tag="mm")
            nc.tensor.matmul(psum_s[:], ones[:], ar_sb[:], start=True, stop=True)

            scale_sb = small.tile([128, 1], f32, name="scale_sb")
            nc.vector.tensor_scalar(
                out=scale_sb[:], in0=psum_s[:],
                scalar1=INV_NELEM, scalar2=1e-5, op0=mult, op1=amax,
            )
            thr = small.tile([128, 1], f32, name="thr")
            nc.vector.tensor_scalar(
                out=thr[:], in0=scale_sb[:], scalar1=0.5, scalar2=None, op0=mult
            )
            nthr = small.tile([128, 1], f32, name="nthr")
            nc.vector.tensor_scalar(
                out=nthr[:], in0=scale_sb[:], scalar1=-0.5, scalar2=None, op0=mult
            )

            # ---- Phase B: quantize fp32 W -> wq in {-1,0,1} fp8, nb-major so
            # pass 1 (nb=0) can start after a quarter of W is quantized.
            def w_consume(i):
                nb, g = divmod(i, QG)
                wst = wtiles.pop(i)
                t1 = tmp_pool.tile([128, 4, 512], f32, name="t1", tag="t1")
                # t1 = (w < -T)
                nc.vector.tensor_scalar(
                    out=t1[:], in0=wst[:],
                    scalar1=nthr[:], scalar2=None,
                    op0=mybir.AluOpType.is_lt,
                )
                # wq = (w > T) - t1
                nc.vector.scalar_tensor_tensor(
                    out=wq[:, 4 * g : 4 * g + 4, nb * 512 : (nb + 1) * 512],
                    in0=wst[:],
                    scalar=thr[:], in1=t1[:],
                    op0=mybir.AluOpType.is_gt, op1=sub,
                )

            for i in range(NQ):
                if i + WLOOK < NQ:
                    w_issue(i + WLOOK)
                w_consume(i)

            # ---- Phase C: out[mb] = x[mb] @ wq.T, fp8 DoubleRow ----
            def x_issue(mb):
                xst = xstage.tile([128, KT, 128], f8, name="xst", tag="xst")
                nc.sync.dma_start(xst[:, :, :], xs[mb])
                rst = rstage.tile([128, KR, 128], f8, name="rst", tag="rst")
                nc.sync.dma_start(rst[:, :, :], rs[mb])
                return xst, rst

            def do_block(mb, nbs, xt):
                xst, rst = xt
                psums = {
                    nb: psum_pool.tile([128, 512], f32, name=f"ps_{mb}_{nb}", tag="mm")
                    for nb in nbs
                }
                for j in range(K2 + K2R):
                    if j < K2:
                        lhs = xst[:, 2 * j : 2 * j + 2, :]
                        jj = j
                    else:
                        jj = j - K2
                        lhs = rst[:, 2 * jj : 2 * jj + 2, :]
                    for nb in nbs:
                        nc.tensor.matmul(
                            psums[nb][:],
                            lhs,
                            wq[:, 2 * jj : 2 * jj + 2, nb * 512 : (nb + 1) * 512],
                            start=(j == 0),
                            stop=(j == K2 + K2R - 1),
                            perf_mode=dr,
                        )
                for nb in nbs:
                    ost = ostage.tile([128, 512], f32, name="ost", tag="ost")
                    # out = psum * scale (fp32), on ScalarE (has a PSUM port)
                    nc.scalar.activation(
                        ost[:],
                        psums[nb][:],
                        mybir.ActivationFunctionType.Copy,
                        scale=scale_sb[:],
                    )
                    nc.gpsimd.dma_start(
                        out[mb * 128 : (mb + 1) * 128, nb * 512 : (nb + 1) * 512],
                        ost[:],
                    )

            XLOOK = 3
            xts = {}
            for p, nbs in ((0, [0]), (1, [1, 2, 3])):
                for mb in range(MB):
                    if not xts:
                        for mb2 in range(mb, min(mb + XLOOK, MB)):
                            xts[(p, mb2)] = x_issue(mb2)
                    look = mb + XLOOK
                    if look < MB:
                        xts[(p, look)] = x_issue(look)
                    elif p == 0 and look - MB < XLOOK:
                        xts[(1, look - MB)] = x_issue(look - MB)
                    do_block(mb, nbs, xts.pop((p, mb)))

    nc.compile()
    return nc


def _get_nc():
    global _nc_cache
    if _nc_cache is None:
        _nc_cache = _build_nc()
    return _nc_cache


def kernel(x: np.ndarray, weight: np.ndarray) -> np.ndarray:
    global LAST_EXEC_NS, LAST_RESULTS
    x = np.asarray(x, dtype=np.float32)
    weight = np.asarray(weight, dtype=np.float32)

    nc = _get_nc()

    f8 = ml_dtypes.float8_e4m3

    # x -> stationary tile layout [mb, k(part), kt, m]: per (mb, p) the
    # [kt, m] plane is contiguous, so each m-block loads as one DMA.
    xf = x.reshape(M, K)
    x8 = xf.astype(f8)
    xsh = np.ascontiguousarray(
        x8.reshape(MB, 128, KT, 128).transpose(0, 3, 2, 1)
    )
    # residual of the fp8 cast, itself in fp8, for the first 7/16 of K
    r = (xf - x8.astype(np.float32))[:, : KR * 128].astype(f8)
    rsh = np.ascontiguousarray(r.reshape(MB, 128, KR, 128).transpose(0, 3, 2, 1))

    wbf = weight.astype(ml_dtypes.bfloat16)

    in_maps = []
    for c in range(N_CORES):
        wsh = weight[c * NL : (c + 1) * NL, :]  # [2048, 4096] fp32
        # quant layout [nb, k(part), kt, n]: per partition the (kt, n) plane
        # is contiguous, so a kt-group loads as one descriptor per partition.
        wtc = np.ascontiguousarray(
            wsh.T.reshape(KT, 128, NB, 512).transpose(2, 1, 0, 3)
        )
        # absum layout: any partitioning works, plain reshape is free
        wac = wbf[c * NL : (c + 1) * NL, :].reshape(AT, 128, 2048)
        in_maps.append({"xs": xsh, "rs": rsh, "wa": wac, "wt": wtc})

    trace = bool(int(os.environ.get("BASS_KERNEL_TRACE", "0")))
    res = run_bass_kernel_spmd(
        nc, in_maps, core_ids=list(range(N_CORES)), trace=trace
    )
    LAST_EXEC_NS = res.exec_time_ns
    LAST_RESULTS = res

    outs = [np.asarray(res.results[c]["out"]) for c in range(N_CORES)]
    full = np.concatenate(outs, axis=1).reshape(B, S, NF).astype(np.float32)
    return full
